# revision 36
# baseline (speedup 1.0000x reference)
"""Trainium2 Bass kernel for nn_DCT: YCbCr 3x3 channel mix + 8x8 block DCT
(stride 8) + repeated min/max normalization collapsed to a per-channel affine.

Sharding: pure data parallel, batch 32 -> 4 samples on each of 8 NeuronCores.

Device algorithm (per core, per sample, f32):
  - x plane loaded as [128 rows, (ci, band, w)] SBUF tiles (contiguous DMA).
  - M1 (per ci, band, chunk): lhsT = X_chunk [h=(br,i), wchunk=(bw',j)],
    rhs = BDCOL [h=(br,i), (u,br)] block-diag DCT basis.
    out T1 = X^T @ BDCOL = [(bw',j), (u,br)]  (column DCT + implicit transpose).
  - copy T1 PSUM->SBUF (scalar engine).
  - M2 (accumulate over ci): lhsT = T1s, rhs = BDROW3[ci] [(bw',j), (co,v,bw')]
    = block-diag DCT basis scaled by ycbcr[co,ci].
    out T2 = [(u,br), (co,v,bw')] = DCT coeffs of the YCbCr image.
  - affine: out = T2 * s + b, with s/b per (sample, co,u,v) host-precomputed
    from max_/min_ (the fori_loop closed form), expanded to matching tiles.
  - DMA out with 256B contiguous runs.
"""

import math
import sys

import numpy as np

for _p in ("/opt/trn_rl_repo", "/opt/pypackages"):
    if _p not in sys.path:
        sys.path.insert(0, _p)

N = 8
IN_CH = 3
EPS = 1e-6
B_FULL = 32
H = 512
W = 512
NCORES = 8
BPC = B_FULL // NCORES  # samples per core
NBANDS = 4  # 512 rows / 128
NCHUNKS = 4  # 512 cols / 128

_CACHED_NC = None


def _dct_basis(n=N):
    u = np.arange(n)
    i = np.arange(n)
    b = np.cos(np.pi * np.outer(u, i + 0.5) / n)
    c = np.full(n, math.sqrt(2.0 / n))
    c[0] = math.sqrt(1.0 / n)
    return (b * c[:, None]).astype(np.float64)


def _build_const_tiles(ycbcr_w):
    """BDCOL [128,128] and BDROW3 [3,128,384] f32 constant matmul operands."""
    D = _dct_basis()  # D[u, i]
    bdcol = np.zeros((128, 128), np.float32)
    for br in range(16):
        # rows (br,i) = br*8+i ; cols (u,br) = u*16+br ; value D[u,i]
        bdcol[br * 8 : (br + 1) * 8, np.arange(8) * 16 + br] = D.T.astype(np.float32)
    bdrow = np.zeros((3, 128, 384), np.float32)
    y = np.asarray(ycbcr_w, np.float64)  # [co, ci]
    for ci in range(3):
        for co in range(3):
            for bw in range(16):
                # rows (bw,j) = bw*8+j ; cols co*128 + v*16 + bw ; D[v,j]*y[co,ci]
                bdrow[ci, bw * 8 : (bw + 1) * 8, co * 128 + np.arange(8) * 16 + bw] = (
                    D.T * y[co, ci]
                ).astype(np.float32)
    return bdcol, bdrow


def _affine_coeffs(max_, min_):
    """Closed form of t -> (t - min)/d applied B_FULL times: out = s*dct + b."""
    m = np.asarray(max_, np.float32)[..., 0, 0]
    n = np.asarray(min_, np.float32)[..., 0, 0]
    d = (m - n + np.float32(EPS)).astype(np.float64)
    r = 1.0 / d
    s = r**B_FULL
    b = -n.astype(np.float64) * (r * (1.0 - s) / (1.0 - r))
    return s.astype(np.float32), b.astype(np.float32)  # [B, 192]


def _expand_tiles(coef):
    """[B,192] -> [B,128,384]: tile[b, u*16+br, co*128+v*16+bw] = coef[b, co,u,v]."""
    c = coef.reshape(-1, 3, 8, 8)  # [b, co, u, v]
    t = c.transpose(0, 2, 1, 3)  # [b, u, co, v]
    t = np.broadcast_to(t[:, :, None, :, :, None], (c.shape[0], 8, 16, 3, 8, 16))
    return np.ascontiguousarray(t.reshape(-1, 128, 384), np.float32)


def _build_nc():
    import concourse.mybir as mybir
    import concourse.tile as tile
    from concourse import bacc
    from contextlib import ExitStack

    f32 = mybir.dt.float32
    bf16 = mybir.dt.bfloat16
    nc = bacc.Bacc()
    x_t = nc.declare_dram_parameter("x", [BPC, 3, H, W], bf16, isOutput=False)
    sb_t = nc.declare_dram_parameter("sb_tile", [BPC, 2, 128, 2, 384], bf16, isOutput=False)
    bdcol_t = nc.declare_dram_parameter("bdcol", [128, 128], bf16, isOutput=False)
    bdrow_t = nc.declare_dram_parameter("bdrow", [3, 128, 384], bf16, isOutput=False)
    # Permuted output layout: [s, band, p=(u,br), (co, v, w)] — fully
    # contiguous bf16 DMA per sample; host untangles at gather time.
    out_t = nc.declare_dram_parameter("out", [BPC, 4, 128, 1536], bf16, isOutput=True)

    # DRAM views
    x_v = x_t[:].rearrange("s c (bd p) w -> s p c bd w", p=128)  # [BPC,128,3,4,512]

    with ExitStack() as ctx:
        tc = ctx.enter_context(tile.TileContext(nc))
        consts = ctx.enter_context(tc.tile_pool(name="consts", bufs=1))
        xp = ctx.enter_context(tc.tile_pool(name="xp", bufs=BPC))
        sbp = ctx.enter_context(tc.tile_pool(name="sbp", bufs=BPC))
        t1sb = ctx.enter_context(tc.tile_pool(name="t1sb", bufs=4 * 3 * BPC))
        t1ps = ctx.enter_context(tc.tile_pool(name="t1ps", bufs=4, space="PSUM"))
        t2ps = ctx.enter_context(tc.tile_pool(name="t2ps", bufs=2, space="PSUM"))
        outp = ctx.enter_context(tc.tile_pool(name="outp", bufs=2))

        bdcol = consts.tile([128, 128], bf16)
        nc.gpsimd.dma_start(out=bdcol, in_=bdcol_t[:])
        bdrow = consts.tile([128, 3, 384], bf16)
        nc.gpsimd.dma_start(out=bdrow, in_=bdrow_t[:].rearrange("c p n -> p c n"))

        for s in range(BPC):
            x_sb = xp.tile([128, 3, NBANDS, W], bf16)
            nc.sync.dma_start(out=x_sb, in_=x_v[s])
            sb_sb = sbp.tile([128, 2, 2, 384], bf16)
            nc.gpsimd.dma_start(
                out=sb_sb,
                in_=sb_t[s].rearrange("t p q n -> p t q n"),
            )
            s_sb = sb_sb[:, 0]
            b_sb = sb_sb[:, 1]

            # One output tile for the whole sample -> single contiguous 3MB DMA
            # free layout (r, chunk, co, v, bw') so per-chunk affine is contiguous
            out_sample = outp.tile([128, NBANDS, NCHUNKS, 3, 8, 16], bf16)
            for r in range(NBANDS):
                t1s_band = []
                for ci in range(3):
                    # 4 chunks' column-DCT into one PSUM bank, one big copy out
                    t1p = t1ps.tile([128, NCHUNKS, 128], f32)
                    for c in range(NCHUNKS):
                        nc.tensor.matmul(
                            t1p[:, c],
                            lhsT=x_sb[:, ci, r, c * 128 : (c + 1) * 128],
                            rhs=bdcol,
                            start=True,
                            stop=True,
                        )
                    t1s = t1sb.tile([128, NCHUNKS, 128], bf16)
                    nc.scalar.copy(out=t1s, in_=t1p)
                    t1s_band.append(t1s)
                for cc in range(2):
                    # two chunks share one (bank-padded) PSUM tile; affine
                    # then runs once over [128, 2, 384]
                    t2p = t2ps.tile([128, 2, 512], f32)
                    for c2 in range(2):
                        for ci in range(3):
                            nc.tensor.matmul(
                                t2p[:, c2, 0:384],
                                lhsT=t1s_band[ci][:, cc * 2 + c2],
                                rhs=bdrow[:, ci],
                                start=(ci == 0),
                                stop=(ci == 2),
                            )
                    dst = out_sample[:, r, cc * 2 : cc * 2 + 2].rearrange("p q c v b -> p q (c v b)")
                    nc.vector.tensor_mul(out=dst, in0=t2p[:, :, 0:384], in1=s_sb)
                    nc.gpsimd.tensor_add(out=dst, in0=dst, in1=b_sb)
            nc.scalar.dma_start(
                out=out_t[s].rearrange("r p f -> p r f"),
                in_=out_sample.rearrange("p r ch c v b -> p r (ch c v b)"),
            )
    return nc


def _get_nc():
    global _CACHED_NC
    if _CACHED_NC is None:
        nc = _build_nc()
        if not nc.is_finalized():
            nc.finalize()  # Bacc: runs compile() (multi-wait splitting etc)
        _CACHED_NC = nc
    return _CACHED_NC


def _make_in_maps(x, max_, min_, ycbcr_w):
    import ml_dtypes

    bf16 = ml_dtypes.bfloat16
    x16 = np.asarray(x, np.float32).astype(bf16)
    s, b = _affine_coeffs(max_, min_)  # [32, 192]
    sb_tiles = np.stack([_expand_tiles(s), _expand_tiles(b)], axis=1)  # [B,2,128,384]
    sb_tiles = np.broadcast_to(
        sb_tiles[:, :, :, None, :], (sb_tiles.shape[0], 2, 128, 2, 384)
    )
    sb_tiles = np.ascontiguousarray(sb_tiles).astype(bf16)
    bdcol, bdrow = _build_const_tiles(np.asarray(ycbcr_w, np.float32))
    bdcol = bdcol.astype(bf16)
    bdrow = bdrow.astype(bf16)

    in_maps = []
    for core in range(NCORES):
        sl = slice(core * BPC, (core + 1) * BPC)
        in_maps.append(
            {
                "x": np.ascontiguousarray(x16[sl]),
                "sb_tile": sb_tiles[sl],
                "bdcol": bdcol,
                "bdrow": bdrow,
            }
        )
    return in_maps


def kernel(x, max_, min_, ycbcr_w, dct_w):
    from concourse.bass_utils import run_bass_kernel_spmd

    nc = _get_nc()
    in_maps = _make_in_maps(x, max_, min_, ycbcr_w)
    res = run_bass_kernel_spmd(nc, in_maps, core_ids=list(range(NCORES)))
    out = np.concatenate([res.results[i]["out"] for i in range(NCORES)], axis=0)
    return _untangle(out)


def _untangle(dev_out):
    """[B, 4, 128, 1536] device layout -> [B, 192, 64, 64] canonical f32."""
    v = np.asarray(dev_out).astype(np.float32)
    v = v.reshape(-1, 4, 8, 16, 4, 3, 8, 16)  # s, r, u, br, c, co, v, bw
    v = v.transpose(0, 5, 2, 6, 1, 3, 4, 7)  # s, co, u, v, r, br, c, bw
    return np.ascontiguousarray(v.reshape(-1, 192, 64, 64))


# revision 37
# speedup vs baseline: 1.2322x; 1.2322x over previous
"""Trainium2 Bass kernel for nn_DCT: YCbCr 3x3 channel mix + 8x8 block DCT
(stride 8) + repeated min/max normalization collapsed to a per-channel affine.

Sharding: pure data parallel, batch 32 -> 4 samples on each of 8 NeuronCores.

Device algorithm (per core, per sample, f32):
  - x plane loaded as [128 rows, (ci, band, w)] SBUF tiles (contiguous DMA).
  - M1 (per ci, band, chunk): lhsT = X_chunk [h=(br,i), wchunk=(bw',j)],
    rhs = BDCOL [h=(br,i), (u,br)] block-diag DCT basis.
    out T1 = X^T @ BDCOL = [(bw',j), (u,br)]  (column DCT + implicit transpose).
  - copy T1 PSUM->SBUF (scalar engine).
  - M2 (accumulate over ci): lhsT = T1s, rhs = BDROW3[ci] [(bw',j), (co,v,bw')]
    = block-diag DCT basis scaled by ycbcr[co,ci].
    out T2 = [(u,br), (co,v,bw')] = DCT coeffs of the YCbCr image.
  - affine: out = T2 * s + b, with s/b per (sample, co,u,v) host-precomputed
    from max_/min_ (the fori_loop closed form), expanded to matching tiles.
  - DMA out with 256B contiguous runs.
"""

import math
import sys

import numpy as np

for _p in ("/opt/trn_rl_repo", "/opt/pypackages"):
    if _p not in sys.path:
        sys.path.insert(0, _p)

N = 8
IN_CH = 3
EPS = 1e-6
B_FULL = 32
H = 512
W = 512
NCORES = 8
BPC = B_FULL // NCORES  # samples per core
NBANDS = 4  # 512 rows / 128
NCHUNKS = 4  # 512 cols / 128

_CACHED_NC = None


def _dct_basis(n=N):
    u = np.arange(n)
    i = np.arange(n)
    b = np.cos(np.pi * np.outer(u, i + 0.5) / n)
    c = np.full(n, math.sqrt(2.0 / n))
    c[0] = math.sqrt(1.0 / n)
    return (b * c[:, None]).astype(np.float64)


def _build_const_tiles(ycbcr_w):
    """BDCOL [128,128] and BDROW3 [3,128,384] f32 constant matmul operands."""
    D = _dct_basis()  # D[u, i]
    bdcol = np.zeros((128, 128), np.float32)
    for br in range(16):
        # rows (br,i) = br*8+i ; cols (u,br) = u*16+br ; value D[u,i]
        bdcol[br * 8 : (br + 1) * 8, np.arange(8) * 16 + br] = D.T.astype(np.float32)
    bdrow = np.zeros((3, 128, 384), np.float32)
    y = np.asarray(ycbcr_w, np.float64)  # [co, ci]
    for ci in range(3):
        for co in range(3):
            for bw in range(16):
                # rows (bw,j) = bw*8+j ; cols co*128 + v*16 + bw ; D[v,j]*y[co,ci]
                bdrow[ci, bw * 8 : (bw + 1) * 8, co * 128 + np.arange(8) * 16 + bw] = (
                    D.T * y[co, ci]
                ).astype(np.float32)
    return bdcol, bdrow


def _affine_coeffs(max_, min_):
    """Closed form of t -> (t - min)/d applied B_FULL times: out = s*dct + b."""
    m = np.asarray(max_, np.float32)[..., 0, 0]
    n = np.asarray(min_, np.float32)[..., 0, 0]
    d = (m - n + np.float32(EPS)).astype(np.float64)
    r = 1.0 / d
    s = r**B_FULL
    b = -n.astype(np.float64) * (r * (1.0 - s) / (1.0 - r))
    return s.astype(np.float32), b.astype(np.float32)  # [B, 192]


def _expand_tiles(coef):
    """[B,192] -> [B,128,384]: tile[b, u*16+br, co*128+v*16+bw] = coef[b, co,u,v]."""
    c = coef.reshape(-1, 3, 8, 8)  # [b, co, u, v]
    t = c.transpose(0, 2, 1, 3)  # [b, u, co, v]
    t = np.broadcast_to(t[:, :, None, :, :, None], (c.shape[0], 8, 16, 3, 8, 16))
    return np.ascontiguousarray(t.reshape(-1, 128, 384), np.float32)


def _build_nc():
    import concourse.mybir as mybir
    import concourse.tile as tile
    from concourse import bacc
    from contextlib import ExitStack

    f32 = mybir.dt.float32
    bf16 = mybir.dt.bfloat16
    nc = bacc.Bacc()
    x_t = nc.declare_dram_parameter("x", [BPC, 3, H, W], bf16, isOutput=False)
    sb_t = nc.declare_dram_parameter("sb_tile", [BPC, 2, 128, 2, 384], bf16, isOutput=False)
    bdcol_t = nc.declare_dram_parameter("bdcol", [128, 128], bf16, isOutput=False)
    bdrow_t = nc.declare_dram_parameter("bdrow", [3, 128, 384], bf16, isOutput=False)
    # Permuted output layout: [s, band, p=(u,br), (co, v, w)] — fully
    # contiguous bf16 DMA per sample; host untangles at gather time.
    out_t = nc.declare_dram_parameter("out", [BPC, 4, 128, 1536], bf16, isOutput=True)

    # DRAM views
    x_v = x_t[:].rearrange("s c (bd p) w -> s p c bd w", p=128)  # [BPC,128,3,4,512]

    with ExitStack() as ctx:
        tc = ctx.enter_context(tile.TileContext(nc))
        consts = ctx.enter_context(tc.tile_pool(name="consts", bufs=1))
        xp = ctx.enter_context(tc.tile_pool(name="xp", bufs=BPC))
        sbp = ctx.enter_context(tc.tile_pool(name="sbp", bufs=BPC))
        t1sb = ctx.enter_context(tc.tile_pool(name="t1sb", bufs=4 * 3 * BPC))
        t1ps = ctx.enter_context(tc.tile_pool(name="t1ps", bufs=4, space="PSUM"))
        t2ps = ctx.enter_context(tc.tile_pool(name="t2ps", bufs=2, space="PSUM"))
        outp = ctx.enter_context(tc.tile_pool(name="outp", bufs=2))

        bdcol = consts.tile([128, 128], bf16)
        nc.gpsimd.dma_start(out=bdcol, in_=bdcol_t[:])
        bdrow = consts.tile([128, 3, 384], bf16)
        nc.gpsimd.dma_start(out=bdrow, in_=bdrow_t[:].rearrange("c p n -> p c n"))

        for s in range(BPC):
            x_sb = xp.tile([128, 3, NBANDS, W], bf16)
            nc.sync.dma_start(out=x_sb, in_=x_v[s])
            sb_sb = sbp.tile([128, 2, 2, 384], bf16)
            nc.gpsimd.dma_start(
                out=sb_sb,
                in_=sb_t[s].rearrange("t p q n -> p t q n"),
            )
            s_sb = sb_sb[:, 0]
            b_sb = sb_sb[:, 1]

            # One output tile for the whole sample -> single contiguous 3MB DMA
            # free layout (r, chunk, co, v, bw') so per-chunk affine is contiguous
            out_sample = outp.tile([128, NBANDS, NCHUNKS, 3, 8, 16], bf16)
            for r in range(NBANDS):
                t1s_band = []
                for ci in range(3):
                    # 4 chunks' column-DCT into one PSUM bank, one big copy out
                    t1p = t1ps.tile([128, NCHUNKS, 128], f32)
                    for c in range(NCHUNKS):
                        nc.tensor.matmul(
                            t1p[:, c],
                            lhsT=x_sb[:, ci, r, c * 128 : (c + 1) * 128],
                            rhs=bdcol,
                            start=True,
                            stop=True,
                        )
                    t1s = t1sb.tile([128, NCHUNKS, 128], bf16)
                    nc.scalar.copy(out=t1s, in_=t1p)
                    t1s_band.append(t1s)
                for cc in range(2):
                    # two chunks share one (bank-padded) PSUM tile; affine
                    # then runs once over [128, 2, 384]
                    t2p = t2ps.tile([128, 2, 512], f32)
                    for c2 in range(2):
                        for ci in range(3):
                            nc.tensor.matmul(
                                t2p[:, c2, 0:384],
                                lhsT=t1s_band[ci][:, cc * 2 + c2],
                                rhs=bdrow[:, ci],
                                start=(ci == 0),
                                stop=(ci == 2),
                            )
                    dst = out_sample[:, r, cc * 2 : cc * 2 + 2].rearrange("p q c v b -> p q (c v b)")
                    nc.vector.tensor_mul(out=dst, in0=t2p[:, :, 0:384], in1=s_sb)
                    nc.vector.tensor_add(out=dst, in0=dst, in1=b_sb)
                nc.scalar.dma_start(
                    out=out_t[s, r],
                    in_=out_sample[:, r].rearrange("p ch c v b -> p (ch c v b)"),
                )
    return nc


def _get_nc():
    global _CACHED_NC
    if _CACHED_NC is None:
        nc = _build_nc()
        if not nc.is_finalized():
            nc.finalize()  # Bacc: runs compile() (multi-wait splitting etc)
        _CACHED_NC = nc
    return _CACHED_NC


def _make_in_maps(x, max_, min_, ycbcr_w):
    import ml_dtypes

    bf16 = ml_dtypes.bfloat16
    x16 = np.asarray(x, np.float32).astype(bf16)
    s, b = _affine_coeffs(max_, min_)  # [32, 192]
    sb_tiles = np.stack([_expand_tiles(s), _expand_tiles(b)], axis=1)  # [B,2,128,384]
    sb_tiles = np.broadcast_to(
        sb_tiles[:, :, :, None, :], (sb_tiles.shape[0], 2, 128, 2, 384)
    )
    sb_tiles = np.ascontiguousarray(sb_tiles).astype(bf16)
    bdcol, bdrow = _build_const_tiles(np.asarray(ycbcr_w, np.float32))
    bdcol = bdcol.astype(bf16)
    bdrow = bdrow.astype(bf16)

    in_maps = []
    for core in range(NCORES):
        sl = slice(core * BPC, (core + 1) * BPC)
        in_maps.append(
            {
                "x": np.ascontiguousarray(x16[sl]),
                "sb_tile": sb_tiles[sl],
                "bdcol": bdcol,
                "bdrow": bdrow,
            }
        )
    return in_maps


def kernel(x, max_, min_, ycbcr_w, dct_w):
    from concourse.bass_utils import run_bass_kernel_spmd

    nc = _get_nc()
    in_maps = _make_in_maps(x, max_, min_, ycbcr_w)
    res = run_bass_kernel_spmd(nc, in_maps, core_ids=list(range(NCORES)))
    out = np.concatenate([res.results[i]["out"] for i in range(NCORES)], axis=0)
    return _untangle(out)


def _untangle(dev_out):
    """[B, 4, 128, 1536] device layout -> [B, 192, 64, 64] canonical f32."""
    v = np.asarray(dev_out).astype(np.float32)
    v = v.reshape(-1, 4, 8, 16, 4, 3, 8, 16)  # s, r, u, br, c, co, v, bw
    v = v.transpose(0, 5, 2, 6, 1, 3, 4, 7)  # s, co, u, v, r, br, c, bw
    return np.ascontiguousarray(v.reshape(-1, 192, 64, 64))
